# revision 14
# baseline (speedup 1.0000x reference)
"""AxialCrossMamba Trainium2 kernel.

Sharding: 8 cores = 4 directions x 2 batch-halves. Each core runs one
direction's Mamba block (its own weights) over two batches. Host does the
direction permutations (row/col/diag/anti, c-major [C, L] token layouts),
and the final 4-direction sigmoid gate.

Device pipeline per (job = one batch):
  in-proj matmul (bf16 PE) -> causal depthwise conv (DVE) -> silu (ACT)
  -> x-proj/dt matmuls (PE) -> softplus w/ fp32 bias (ACT)
  -> selective scan: a = exp(dt*A) per-state-column ACT activations (fp32),
     b = u*B (bf16), tensor_tensor_scan over flattened (s,t) with
     boundary-reset columns, h*C + strided reduce over s
  -> y = ys + xs*D, gate silu(z), out-proj matmul (bf16 PE).
"""

import sys

for _p in ("/opt/trn_rl_repo", "/root/.axon_site/_ro/trn_rl_repo"):
    if _p not in sys.path:
        sys.path.insert(0, _p)

from contextlib import ExitStack

import numpy as np
import ml_dtypes

import concourse.bass as bass
from concourse import bacc
import concourse.mybir as mybir
import concourse.tile as tile
from concourse.bass_utils import run_bass_kernel_spmd

BF16 = ml_dtypes.bfloat16

# Problem constants
B_, C_, H_, W_ = 4, 192, 64, 64
L = H_ * W_          # 4096 tokens
DS, DC = 16, 4       # d_state, d_conv
DI = 2 * C_          # 384 d_inner
DTR = (C_ + 15) // 16  # 12 dt_rank
NB = 2               # batches per core
ND = DI // 128       # 3 d-blocks
N_CORES = 8

GP_SCAN = False

AF = mybir.ActivationFunctionType
ALU = mybir.AluOpType
FP32 = mybir.dt.float32
BF = mybir.dt.bfloat16


def build_nc(L=L, TC=512, SB=4):
    """Build the SPMD single-core program (identical on all 8 cores)."""
    nc = bacc.Bacc("TRN2", debug=False)

    # ---- DRAM I/O ----
    tokT = nc.dram_tensor("tokT", [NB, C_, L], BF, kind="ExternalInput").ap()
    Win = nc.dram_tensor("Win", [C_, 2 * DI], BF, kind="ExternalInput").ap()
    convw = nc.dram_tensor("convw", [DI, DC], FP32, kind="ExternalInput").ap()
    convb = nc.dram_tensor("convb", [DI, 1], FP32, kind="ExternalInput").ap()
    Wx = nc.dram_tensor("Wx", [DI, 96], BF, kind="ExternalInput").ap()
    Wdt = nc.dram_tensor("Wdt", [DTR, DI], BF, kind="ExternalInput").ap()
    bdt = nc.dram_tensor("bdt", [DI, 1], FP32, kind="ExternalInput").ap()
    Acoef = nc.dram_tensor("Acoef", [DI, DS], FP32, kind="ExternalInput").ap()
    Dsk = nc.dram_tensor("Dsk", [DI, 1], FP32, kind="ExternalInput").ap()
    Wout = nc.dram_tensor("Wout", [DI, C_], BF, kind="ExternalInput").ap()
    outT = nc.dram_tensor("outT", [NB, C_, L], FP32, kind="ExternalOutput").ap()
    # scratch
    z_scr = nc.dram_tensor("z_scr", [NB, ND, 128, L], BF, kind="Internal").ap()
    y_scr = nc.dram_tensor("y_scr", [NB, ND, 128, L], BF, kind="Internal").ap()
    bc_scr = nc.dram_tensor("bc_scr", [NB, 2, L // TC, DS * TC], BF, kind="Internal").ap()

    io = dict(tokT=tokT, Win=Win, convw=convw, convb=convb, Wx=Wx, Wdt=Wdt,
              bdt=bdt, Acoef=Acoef, Dsk=Dsk, Wout=Wout, outT=outT,
              z_scr=z_scr, y_scr=y_scr, bc_scr=bc_scr)
    with tile.TileContext(nc) as tc:
        with ExitStack() as ctx:
            _emit(ctx, tc, nc, io, L=L, TC=TC, SB=SB)
    nc.compile()
    return nc


def _emit(ctx, tc, nc, io, *, L, TC, SB):
    tokT, Win, convw, convb, Wx, Wdt, bdt = (
        io["tokT"], io["Win"], io["convw"], io["convb"], io["Wx"], io["Wdt"],
        io["bdt"])
    Acoef, Dsk, Wout, outT = io["Acoef"], io["Dsk"], io["Wout"], io["outT"]
    z_scr, y_scr, bc_scr = io["z_scr"], io["y_scr"], io["bc_scr"]

    P = 128
    NCH = L // TC          # t-chunks
    NSB = DS // SB         # s-blocks
    NN = max(1, L // 512)  # matmul n-chunks
    NSZ = L // NN

    # ---- pools ----
    wp = ctx.enter_context(tc.tile_pool(name="weights", bufs=1))
    big = ctx.enter_context(tc.tile_pool(name="big", bufs=4))    # bf16 [128,L]
    fxf = ctx.enter_context(tc.tile_pool(name="fxf", bufs=1))    # fp32 conv acc
    af32 = ctx.enter_context(tc.tile_pool(name="af32", bufs=2))  # fp32 scan a
    hbf = ctx.enter_context(tc.tile_pool(name="hbf", bufs=2))    # bf16 scan h
    bcls = ctx.enter_context(tc.tile_pool(name="bcls", bufs=6))  # bf16 scan misc
    dtp = ctx.enter_context(tc.tile_pool(name="dtp", bufs=1))    # dt bf16 resident
    xsp = ctx.enter_context(tc.tile_pool(name="xsp", bufs=1))    # xs bf16 resident
    sm = ctx.enter_context(tc.tile_pool(name="small", bufs=2))
    smE = ctx.enter_context(tc.tile_pool(name="smallE", bufs=2))
    pp = ctx.enter_context(tc.tile_pool(name="psum", bufs=4, space="PSUM"))
    pp2 = ctx.enter_context(tc.tile_pool(name="psum2", bufs=2, space="PSUM"))

    # ---- load weights ----
    win0 = wp.tile([P, 2 * DI], BF, tag="win0")
    win1 = wp.tile([C_ - P, 2 * DI], BF, tag="win1")
    nc.sync.dma_start(win0[:], Win[0:P, :])
    nc.sync.dma_start(win1[:], Win[P:C_, :])
    wdt_full = wp.tile([DTR, DI], BF, tag="wdt")
    nc.sync.dma_start(wdt_full[:], Wdt[:])
    wxs, cw3, cb3, bdt3, ac3, dsk3, wo3 = [], [], [], [], [], [], []
    for db in range(ND):
        r = slice(db * P, (db + 1) * P)
        w1 = wp.tile([P, 96], BF, tag=f"wx{db}")
        nc.sync.dma_start(w1[:], Wx[r, :]); wxs.append(w1)
        w2 = wp.tile([P, DC], FP32, tag=f"cw{db}")
        nc.sync.dma_start(w2[:], convw[r, :]); cw3.append(w2)
        w3 = wp.tile([P, 1], FP32, tag=f"cb{db}")
        nc.sync.dma_start(w3[:], convb[r, :]); cb3.append(w3)
        w4 = wp.tile([P, 1], FP32, tag=f"bdt{db}")
        nc.sync.dma_start(w4[:], bdt[r, :]); bdt3.append(w4)
        w5 = wp.tile([P, DS], FP32, tag=f"ac{db}")
        nc.sync.dma_start(w5[:], Acoef[r, :]); ac3.append(w5)
        w6 = wp.tile([P, 1], FP32, tag=f"dsk{db}")
        nc.sync.dma_start(w6[:], Dsk[r, :]); dsk3.append(w6)
        w7 = wp.tile([P, C_], BF, tag=f"wo{db}")
        nc.sync.dma_start(w7[:], Wout[r, :]); wo3.append(w7)

    for j in range(NB):
        # ================= A: in-proj (+ conv interleaved) =================
        tok0 = big.tile([P, L], BF, tag="big")
        tok1 = big.tile([C_ - P, L], BF, tag="big")
        nc.sync.dma_start(tok0[:], tokT[j, 0:P, :])
        nc.sync.dma_start(tok1[:], tokT[j, P:C_, :])

        xs = []
        for m in range(2 * DI // P):   # M-blocks of xz^T; 0..2 -> xi, 3..5 -> z
            if m < ND:
                xi = big.tile([P, L + DC], BF, tag="big")
                nc.scalar.memzero(xi[:, 0:DC])
            mm = slice(m * P, (m + 1) * P)
            for n in range(NN):
                ns = slice(n * NSZ, (n + 1) * NSZ)
                ps = pp.tile([P, NSZ], FP32, tag="ps")
                nc.tensor.matmul(ps[:], win0[:, mm], tok0[:, ns],
                                 start=True, stop=False)
                nc.tensor.matmul(ps[:], win1[:, mm], tok1[:, ns],
                                 start=False, stop=True)
                if m < ND:
                    nc.scalar.copy(xi[:, DC + n * NSZ: DC + (n + 1) * NSZ],
                                   ps[:])
                else:
                    zt = smE.tile([P, NSZ], BF, tag="ztmp")
                    nc.vector.tensor_copy(zt[:], ps[:])
                    nc.sync.dma_start(z_scr[j, m - ND, :, ns], zt[:])
            if m < ND:
                # conv + silu for this d-block; frees xi slot for the next one
                db = m
                x_ = xsp.tile([P, L], BF, tag=f"xs{db}")
                xf = fxf.tile([P, L], FP32, tag="xf")
                # xc[t] = sum_k w[k]*tok[t-3+k]; token t sits at xi col t+DC
                nc.vector.tensor_scalar_mul(xf[:], xi[:, 1:1 + L],
                                            cw3[db][:, 0:1])
                for k in range(1, DC):
                    nc.vector.scalar_tensor_tensor(
                        xf[:], xi[:, 1 + k:1 + k + L], cw3[db][:, k:k + 1],
                        xf[:], ALU.mult, ALU.add)
                sgt = big.tile([P, L], BF, tag="big")
                nc.scalar.activation(sgt[:], xf[:], AF.Sigmoid, bias=cb3[db])
                nc.vector.scalar_tensor_tensor(x_[:], xf[:], cb3[db][:],
                                               sgt[:], ALU.add, ALU.mult)
                xs.append(x_)

        # ================= C: dbc, dt =================
        dtl = bcls.tile([DTR, L], BF, tag="dtl", bufs=1)
        for n in range(NN):
            ns = slice(n * NSZ, (n + 1) * NSZ)
            psd = pp2.tile([96, NSZ], FP32, tag="psd")
            for db in range(ND):
                nc.tensor.matmul(psd[:], wxs[db][:], xs[db][:, ns],
                                 start=(db == 0), stop=(db == ND - 1))
            nc.scalar.copy(dtl[:, ns], psd[0:DTR, :])
            bt = smE.tile([DS, NSZ], BF, tag="bct")
            ct = smE.tile([DS, NSZ], BF, tag="bct")
            nc.vector.tensor_copy(bt[:], psd[32:32 + DS, :])
            nc.vector.tensor_copy(ct[:], psd[64:64 + DS, :])
            for r in range(max(1, NSZ // TC)):
                rs = slice(r * TC, (r + 1) * TC)
                nc.sync.dma_start(
                    bc_scr[j, 0, n * (NSZ // TC) + r]
                    .rearrange("(s t) -> s t", s=DS), bt[:, rs])
                nc.sync.dma_start(
                    bc_scr[j, 1, n * (NSZ // TC) + r]
                    .rearrange("(s t) -> s t", s=DS), ct[:, rs])
        dtf = []
        for db in range(ND):
            d_ = dtp.tile([P, L], BF, tag=f"dt{db}")
            esp = fxf.tile([P, L], FP32, tag="xf")
            for n in range(NN):
                ns = slice(n * NSZ, (n + 1) * NSZ)
                psm = pp.tile([P, NSZ], FP32, tag="ps")
                nc.tensor.matmul(psm[:], wdt_full[:, db * P:(db + 1) * P],
                                 dtl[:, ns], start=True, stop=True)
                nc.scalar.activation(esp[:, ns], psm[:], AF.Exp, bias=bdt3[db])
            nc.scalar.activation(d_[:], esp[:], AF.Ln, bias=1.0)
            dtf.append(d_)

        # ================= D: selective scan =================
        hcarry = {}
        for db in range(ND):
            for sb in range(NSB):
                hcarry[(db, sb)] = sm.tile([P, SB, 1], FP32, name=f"carry{db}{sb}",
                                           tag=f"carry{db}_{sb}", bufs=1)
        for ch in range(NCH):
            cs = slice(ch * TC, (ch + 1) * TC)
            for db in range(ND):
                uch = sm.tile([P, TC], BF, tag="uch")
                nc.gpsimd.tensor_tensor(uch[:], dtf[db][:, cs], xs[db][:, cs],
                                        ALU.mult)
                uv = uch[:].unsqueeze(1).broadcast_to((P, SB, TC))
                ysbs = []
                for sb in range(NSB):
                    seng = nc.gpsimd if (GP_SCAN and sb % 2 == 1) else nc.vector
                    brep = bcls.tile([P, SB, TC], BF, tag="bcls")
                    crep = bcls.tile([P, SB, TC], BF, tag="bcls")
                    nc.sync.dma_start(
                        brep[:],
                        bc_scr[j, 0, ch, sb * SB * TC:(sb + 1) * SB * TC]
                        .rearrange("(s t) -> s t", s=SB)
                        .unsqueeze(0).broadcast_to((P, SB, TC)))
                    nc.sync.dma_start(
                        crep[:],
                        bc_scr[j, 1, ch, sb * SB * TC:(sb + 1) * SB * TC]
                        .rearrange("(s t) -> s t", s=SB)
                        .unsqueeze(0).broadcast_to((P, SB, TC)))
                    a_ = af32.tile([P, SB, TC + 1], FP32, tag="a")
                    for s8 in range(SB):
                        s = sb * SB + s8
                        nc.scalar.activation(a_[:, s8, 1:], dtf[db][:, cs],
                                             AF.Exp, scale=ac3[db][:, s:s + 1])
                    nc.scalar.memzero(a_[:, :, 0:1])
                    b_ = bcls.tile([P, SB, TC + 1], BF, tag="bcls")
                    nc.vector.tensor_tensor(b_[:, :, 1:], uv, brep[:],
                                            ALU.mult)
                    if ch == 0:
                        nc.gpsimd.memset(b_[:, :, 0:1], 0.0)
                    else:
                        nc.gpsimd.tensor_copy(b_[:, :, 0:1],
                                              hcarry[(db, sb)][:])
                    h_ = hbf.tile([P, SB, TC + 1], BF, tag="h")
                    seng.tensor_tensor_scan(
                        h_[:].rearrange("p s t -> p (s t)"),
                        a_[:].rearrange("p s t -> p (s t)"),
                        b_[:].rearrange("p s t -> p (s t)"),
                        0.0, ALU.mult, ALU.add)
                    nc.gpsimd.tensor_copy(hcarry[(db, sb)][:],
                                          h_[:, :, TC:TC + 1])
                    hcm = bcls.tile([P, SB, TC], BF, tag="bcls")
                    heng = nc.gpsimd if sb % 2 == 1 else nc.vector
                    heng.tensor_tensor(hcm[:], h_[:, :, 1:], crep[:],
                                       ALU.mult)
                    # pairwise tree-sum over the SB states (contiguous, 2x)
                    t2 = sm.tile([P, 2, TC], BF, tag="t2")
                    nc.vector.tensor_tensor(t2[:], hcm[:, 0:2, :],
                                            hcm[:, 2:4, :], ALU.add)
                    ysb = sm.tile([P, TC], BF, tag=f"ysb{sb}")
                    nc.vector.tensor_tensor(ysb[:], t2[:, 0, :], t2[:, 1, :],
                                            ALU.add)
                    ysbs.append(ysb)
                y01 = sm.tile([P, TC], BF, tag="y01")
                nc.vector.tensor_tensor(y01[:], ysbs[0][:], ysbs[1][:], ALU.add)
                y23 = sm.tile([P, TC], BF, tag="y23")
                nc.vector.tensor_tensor(y23[:], ysbs[2][:], ysbs[3][:], ALU.add)
                y0 = sm.tile([P, TC], FP32, tag="y0")
                nc.vector.tensor_tensor(y0[:], y01[:], y23[:], ALU.add)
                # y = ys + xs*D -> bf16 -> DRAM
                nc.vector.scalar_tensor_tensor(y0[:], xs[db][:, cs],
                                               dsk3[db][:], y0[:],
                                               ALU.mult, ALU.add)
                ybf = sm.tile([P, TC], BF, tag="ybf")
                nc.scalar.copy(ybf[:], y0[:])
                nc.sync.dma_start(y_scr[j, db, :, cs], ybf[:])

    # ================= E: gate + out-proj =================
    for j in range(NB):
        for n in range(NN):
            ns = slice(n * NSZ, (n + 1) * NSZ)
            ygs = []
            for db in range(ND):
                zt = smE.tile([P, NSZ], BF, tag="ze", bufs=3)
                nc.sync.dma_start(zt[:], z_scr[j, db, :, ns])
                sgz = smE.tile([P, NSZ], BF, tag="sgz", bufs=3)
                nc.scalar.activation(sgz[:], zt[:], AF.Sigmoid)
                yt = smE.tile([P, NSZ], BF, tag="ye", bufs=3)
                nc.sync.dma_start(yt[:], y_scr[j, db, :, ns])
                nc.gpsimd.tensor_tensor(yt[:], yt[:], zt[:], ALU.mult)
                nc.gpsimd.tensor_tensor(yt[:], yt[:], sgz[:], ALU.mult)
                ygs.append(yt)
            for m in range(2):
                msz = P if m == 0 else C_ - P
                mm = slice(m * P, m * P + msz)
                pso = pp2.tile([msz, NSZ], FP32, tag="pso")
                for db in range(ND):
                    nc.tensor.matmul(pso[:], wo3[db][:, mm], ygs[db][:],
                                     start=(db == 0), stop=(db == ND - 1))
                ot = smE.tile([msz, NSZ], FP32, tag="oe")
                nc.scalar.copy(ot[:], pso[:])
                nc.sync.dma_start(outT[j, mm, ns], ot[:])


# ---------------- host side ----------------

_CACHE = {}
PROFILE = False
PROFILE_KW = {}


def _get_nc():
    if "nc" not in _CACHE:
        _CACHE["nc"] = build_nc()
    return _CACHE["nc"]


def _permute_toks(x, idx):
    """x: [C, H, W] fp32 -> 4 direction token maps, each [C, L] (c-major)."""
    c = x.shape[0]
    row = x.reshape(c, -1)
    col = x.transpose(0, 2, 1).reshape(c, -1)
    diag = row[:, idx]
    anti = x[:, :, ::-1].reshape(c, -1)[:, idx]
    return [row, col, diag, anti]


def _unpermute(outs, inv_idx, h, w):
    """outs: list of 4 [C, L] -> sum of un-permuted direction outputs."""
    c = outs[0].shape[0]
    row_f = outs[0].reshape(c, h, w)
    col_f = outs[1].reshape(c, w, h).transpose(0, 2, 1)
    diag_f = outs[2][:, inv_idx].reshape(c, h, w)
    anti_f = outs[3][:, inv_idx].reshape(c, h, w)[:, :, ::-1]
    return row_f + col_f + diag_f + anti_f


def _pack_wx(wx):
    """Pad W_x columns so dt/B/C rows land at PSUM partitions 0/32/64."""
    out = np.zeros((DI, 96), np.float32)
    out[:, 0:DTR] = wx[:, 0:DTR]
    out[:, 32:32 + DS] = wx[:, DTR:DTR + DS]
    out[:, 64:64 + DS] = wx[:, DTR + DS:]
    return out.astype(BF16)


def kernel(x, W_in, conv_w, conv_b, W_x, W_dt, b_dt, A_log, D_skip, W_out,
           idx, inv_idx):
    x = np.asarray(x, np.float32)
    idx = np.asarray(idx, np.int32)
    inv_idx = np.asarray(inv_idx, np.int32)
    A = -np.exp(np.asarray(A_log, np.float32))        # [4, DI, DS]
    conv_b = np.asarray(conv_b, np.float32)
    b_dt = np.asarray(b_dt, np.float32)
    D_skip = np.asarray(D_skip, np.float32)

    nc = _get_nc()
    in_maps = []
    for core in range(N_CORES):
        d = core // 2      # direction
        bh = core % 2      # batch half
        toks = np.empty((NB, C_, L), BF16)
        for jb in range(NB):
            b = bh * NB + jb
            toks[jb] = _permute_toks(x[b], idx)[d].astype(BF16)
        in_maps.append(dict(
            tokT=toks,
            Win=np.asarray(W_in[d], np.float32).astype(BF16),
            convw=np.ascontiguousarray(np.asarray(conv_w[d], np.float32)),
            convb=np.ascontiguousarray(conv_b[d].reshape(DI, 1)),
            Wx=_pack_wx(np.asarray(W_x[d], np.float32)),
            Wdt=np.asarray(W_dt[d], np.float32).astype(BF16),
            bdt=np.ascontiguousarray(b_dt[d].reshape(DI, 1)),
            Acoef=np.ascontiguousarray(A[d]),
            Dsk=np.ascontiguousarray(D_skip[d].reshape(DI, 1)),
            Wout=np.asarray(W_out[d], np.float32).astype(BF16),
        ))

    res = run_bass_kernel_spmd(nc, in_maps, list(range(N_CORES)),
                               trace=PROFILE, **PROFILE_KW)
    _CACHE["last_exec_ns"] = res.exec_time_ns
    outs = res.results

    # gather: per batch b, the 4 direction outputs live on cores d*2 + b//2
    acc = np.zeros((B_, C_, H_, W_), np.float32)
    for b in range(B_):
        bh, jb = b // NB, b % NB
        douts = [np.asarray(outs[d * 2 + bh]["outT"][jb], np.float32)
                 for d in range(4)]
        acc[b] = _unpermute(douts, inv_idx, H_, W_)
    gate = 1.0 / (1.0 + np.exp(-0.25 * acc))
    return x * gate


# revision 15
# speedup vs baseline: 1.1052x; 1.1052x over previous
"""AxialCrossMamba Trainium2 kernel.

Sharding: 8 cores = 4 directions x 2 batch-halves. Each core runs one
direction's Mamba block (its own weights) over two batches. Host does the
direction permutations (row/col/diag/anti, c-major [C, L] token layouts),
and the final 4-direction sigmoid gate.

Device pipeline per (job = one batch):
  in-proj matmul (bf16 PE) -> causal depthwise conv (DVE) -> silu (ACT)
  -> x-proj/dt matmuls (PE) -> softplus w/ fp32 bias (ACT)
  -> selective scan: a = exp(dt*A) per-state-column ACT activations (fp32),
     b = u*B (bf16), tensor_tensor_scan over flattened (s,t) with
     boundary-reset columns, h*C + strided reduce over s
  -> y = ys + xs*D, gate silu(z), out-proj matmul (bf16 PE).
"""

import sys

for _p in ("/opt/trn_rl_repo", "/root/.axon_site/_ro/trn_rl_repo"):
    if _p not in sys.path:
        sys.path.insert(0, _p)

from contextlib import ExitStack

import numpy as np
import ml_dtypes

import concourse.bass as bass
from concourse import bacc
import concourse.mybir as mybir
import concourse.tile as tile
from concourse.bass_utils import run_bass_kernel_spmd

BF16 = ml_dtypes.bfloat16

# Problem constants
B_, C_, H_, W_ = 4, 192, 64, 64
L = H_ * W_          # 4096 tokens
DS, DC = 16, 4       # d_state, d_conv
DI = 2 * C_          # 384 d_inner
DTR = (C_ + 15) // 16  # 12 dt_rank
NB = 2               # batches per core
ND = DI // 128       # 3 d-blocks
N_CORES = 8

GP_SCAN = False

AF = mybir.ActivationFunctionType
ALU = mybir.AluOpType
FP32 = mybir.dt.float32
BF = mybir.dt.bfloat16


def build_nc(L=L, TC=512, SB=4):
    """Build the SPMD single-core program (identical on all 8 cores)."""
    nc = bacc.Bacc("TRN2", debug=False)

    # ---- DRAM I/O ----
    tokT = nc.dram_tensor("tokT", [NB, C_, L], BF, kind="ExternalInput").ap()
    Win = nc.dram_tensor("Win", [C_, 2 * DI], BF, kind="ExternalInput").ap()
    convw = nc.dram_tensor("convw", [DI, DC], FP32, kind="ExternalInput").ap()
    convb = nc.dram_tensor("convb", [DI, 1], FP32, kind="ExternalInput").ap()
    Wx = nc.dram_tensor("Wx", [DI, 96], BF, kind="ExternalInput").ap()
    Wdt = nc.dram_tensor("Wdt", [DTR, DI], BF, kind="ExternalInput").ap()
    bdt = nc.dram_tensor("bdt", [DI, 1], FP32, kind="ExternalInput").ap()
    Acoef = nc.dram_tensor("Acoef", [DI, DS], FP32, kind="ExternalInput").ap()
    Dsk = nc.dram_tensor("Dsk", [DI, 1], FP32, kind="ExternalInput").ap()
    Wout = nc.dram_tensor("Wout", [DI, C_], BF, kind="ExternalInput").ap()
    outT = nc.dram_tensor("outT", [NB, C_, L], FP32, kind="ExternalOutput").ap()
    # scratch
    z_scr = nc.dram_tensor("z_scr", [NB, ND, 128, L], BF, kind="Internal").ap()
    y_scr = nc.dram_tensor("y_scr", [NB, ND, 128, L], BF, kind="Internal").ap()
    bc_scr = nc.dram_tensor("bc_scr", [NB, 2, L // TC, DS * TC], BF, kind="Internal").ap()

    io = dict(tokT=tokT, Win=Win, convw=convw, convb=convb, Wx=Wx, Wdt=Wdt,
              bdt=bdt, Acoef=Acoef, Dsk=Dsk, Wout=Wout, outT=outT,
              z_scr=z_scr, y_scr=y_scr, bc_scr=bc_scr)
    with tile.TileContext(nc) as tc:
        with ExitStack() as ctx:
            _emit(ctx, tc, nc, io, L=L, TC=TC, SB=SB)
    nc.compile()
    return nc


def _emit(ctx, tc, nc, io, *, L, TC, SB):
    tokT, Win, convw, convb, Wx, Wdt, bdt = (
        io["tokT"], io["Win"], io["convw"], io["convb"], io["Wx"], io["Wdt"],
        io["bdt"])
    Acoef, Dsk, Wout, outT = io["Acoef"], io["Dsk"], io["Wout"], io["outT"]
    z_scr, y_scr, bc_scr = io["z_scr"], io["y_scr"], io["bc_scr"]

    P = 128
    NCH = L // TC          # t-chunks
    NSB = DS // SB         # s-blocks
    NN = max(1, L // 512)  # matmul n-chunks
    NSZ = L // NN

    # ---- pools ----
    wp = ctx.enter_context(tc.tile_pool(name="weights", bufs=1))
    big = ctx.enter_context(tc.tile_pool(name="big", bufs=4))    # bf16 [128,L]
    fxf = ctx.enter_context(tc.tile_pool(name="fxf", bufs=1))    # fp32 conv acc
    af32 = ctx.enter_context(tc.tile_pool(name="af32", bufs=2))  # fp32 scan a
    hbf = ctx.enter_context(tc.tile_pool(name="hbf", bufs=2))    # bf16 scan h
    bcls = ctx.enter_context(tc.tile_pool(name="bcls", bufs=6))  # bf16 scan misc
    dtp = ctx.enter_context(tc.tile_pool(name="dtp", bufs=1))    # dt bf16 resident
    xsp = ctx.enter_context(tc.tile_pool(name="xsp", bufs=1))    # xs bf16 resident
    sm = ctx.enter_context(tc.tile_pool(name="small", bufs=2))
    smE = ctx.enter_context(tc.tile_pool(name="smallE", bufs=2))
    pp = ctx.enter_context(tc.tile_pool(name="psum", bufs=4, space="PSUM"))
    pp2 = ctx.enter_context(tc.tile_pool(name="psum2", bufs=2, space="PSUM"))

    # ---- load weights ----
    win0 = wp.tile([P, 2 * DI], BF, tag="win0")
    win1 = wp.tile([C_ - P, 2 * DI], BF, tag="win1")
    nc.sync.dma_start(win0[:], Win[0:P, :])
    nc.sync.dma_start(win1[:], Win[P:C_, :])
    wdt_full = wp.tile([DTR, DI], BF, tag="wdt")
    nc.sync.dma_start(wdt_full[:], Wdt[:])
    wxs, cw3, cb3, bdt3, ac3, dsk3, wo3 = [], [], [], [], [], [], []
    for db in range(ND):
        r = slice(db * P, (db + 1) * P)
        w1 = wp.tile([P, 96], BF, tag=f"wx{db}")
        nc.sync.dma_start(w1[:], Wx[r, :]); wxs.append(w1)
        w2 = wp.tile([P, DC], FP32, tag=f"cw{db}")
        nc.sync.dma_start(w2[:], convw[r, :]); cw3.append(w2)
        w3 = wp.tile([P, 1], FP32, tag=f"cb{db}")
        nc.sync.dma_start(w3[:], convb[r, :]); cb3.append(w3)
        w4 = wp.tile([P, 1], FP32, tag=f"bdt{db}")
        nc.sync.dma_start(w4[:], bdt[r, :]); bdt3.append(w4)
        w5 = wp.tile([P, DS], FP32, tag=f"ac{db}")
        nc.sync.dma_start(w5[:], Acoef[r, :]); ac3.append(w5)
        w6 = wp.tile([P, 1], FP32, tag=f"dsk{db}")
        nc.sync.dma_start(w6[:], Dsk[r, :]); dsk3.append(w6)
        w7 = wp.tile([P, C_], BF, tag=f"wo{db}")
        nc.sync.dma_start(w7[:], Wout[r, :]); wo3.append(w7)

    for j in range(NB):
        # ================= A: in-proj (+ conv interleaved) =================
        tok0 = big.tile([P, L], BF, tag="big")
        tok1 = big.tile([C_ - P, L], BF, tag="big")
        nc.sync.dma_start(tok0[:], tokT[j, 0:P, :])
        nc.sync.dma_start(tok1[:], tokT[j, P:C_, :])

        xs = []
        for m in range(2 * DI // P):   # M-blocks of xz^T; 0..2 -> xi, 3..5 -> z
            if m < ND:
                xi = big.tile([P, L + DC], BF, tag="big")
                nc.scalar.memzero(xi[:, 0:DC])
            mm = slice(m * P, (m + 1) * P)
            for n in range(NN):
                ns = slice(n * NSZ, (n + 1) * NSZ)
                ps = pp.tile([P, NSZ], FP32, tag="ps")
                nc.tensor.matmul(ps[:], win0[:, mm], tok0[:, ns],
                                 start=True, stop=False)
                nc.tensor.matmul(ps[:], win1[:, mm], tok1[:, ns],
                                 start=False, stop=True)
                if m < ND:
                    nc.scalar.copy(xi[:, DC + n * NSZ: DC + (n + 1) * NSZ],
                                   ps[:])
                else:
                    zt = smE.tile([P, NSZ], BF, tag="ztmp")
                    nc.vector.tensor_copy(zt[:], ps[:])
                    nc.sync.dma_start(z_scr[j, m - ND, :, ns], zt[:])
            if m < ND:
                # conv + silu for this d-block; frees xi slot for the next one
                db = m
                x_ = xsp.tile([P, L], BF, tag=f"xs{db}")
                xf = fxf.tile([P, L], FP32, tag="xf")
                # xc[t] = sum_k w[k]*tok[t-3+k]; token t sits at xi col t+DC
                nc.vector.tensor_scalar_mul(xf[:], xi[:, 1:1 + L],
                                            cw3[db][:, 0:1])
                for k in range(1, DC):
                    nc.vector.scalar_tensor_tensor(
                        xf[:], xi[:, 1 + k:1 + k + L], cw3[db][:, k:k + 1],
                        xf[:], ALU.mult, ALU.add)
                sgt = big.tile([P, L], BF, tag="big")
                nc.scalar.activation(sgt[:], xf[:], AF.Sigmoid, bias=cb3[db])
                nc.vector.scalar_tensor_tensor(x_[:], xf[:], cb3[db][:],
                                               sgt[:], ALU.add, ALU.mult)
                xs.append(x_)

        # ================= C: dbc, dt =================
        dtl = bcls.tile([DTR, L], BF, tag="dtl", bufs=1)
        for n in range(NN):
            ns = slice(n * NSZ, (n + 1) * NSZ)
            psd = pp2.tile([96, NSZ], FP32, tag="psd")
            for db in range(ND):
                nc.tensor.matmul(psd[:], wxs[db][:], xs[db][:, ns],
                                 start=(db == 0), stop=(db == ND - 1))
            nc.scalar.copy(dtl[:, ns], psd[0:DTR, :])
            bt = smE.tile([DS, NSZ], BF, tag="bct")
            ct = smE.tile([DS, NSZ], BF, tag="bct")
            nc.vector.tensor_copy(bt[:], psd[32:32 + DS, :])
            nc.vector.tensor_copy(ct[:], psd[64:64 + DS, :])
            for r in range(max(1, NSZ // TC)):
                rs = slice(r * TC, (r + 1) * TC)
                nc.sync.dma_start(
                    bc_scr[j, 0, n * (NSZ // TC) + r]
                    .rearrange("(s t) -> s t", s=DS), bt[:, rs])
                nc.sync.dma_start(
                    bc_scr[j, 1, n * (NSZ // TC) + r]
                    .rearrange("(s t) -> s t", s=DS), ct[:, rs])
        dtf = []
        for db in range(ND):
            d_ = dtp.tile([P, L], BF, tag=f"dt{db}")
            esp = fxf.tile([P, L], FP32, tag="xf")
            for n in range(NN):
                ns = slice(n * NSZ, (n + 1) * NSZ)
                psm = pp.tile([P, NSZ], FP32, tag="ps")
                nc.tensor.matmul(psm[:], wdt_full[:, db * P:(db + 1) * P],
                                 dtl[:, ns], start=True, stop=True)
                nc.scalar.activation(esp[:, ns], psm[:], AF.Exp, bias=bdt3[db])
            nc.scalar.activation(d_[:], esp[:], AF.Ln, bias=1.0)
            dtf.append(d_)

        # ================= D: selective scan =================
        hcarry = {}
        for db in range(ND):
            for sb in range(NSB):
                hcarry[(db, sb)] = sm.tile([P, SB, 1], FP32, name=f"carry{db}{sb}",
                                           tag=f"carry{db}_{sb}", bufs=1)
        for ch in range(NCH):
            cs = slice(ch * TC, (ch + 1) * TC)
            for db in range(ND):
                uch = sm.tile([P, TC], BF, tag="uch")
                nc.vector.tensor_tensor(uch[:], dtf[db][:, cs], xs[db][:, cs],
                                        ALU.mult)
                uv = uch[:].unsqueeze(1).broadcast_to((P, SB, TC))
                ysbs = []
                for sb in range(NSB):
                    seng = nc.gpsimd if (GP_SCAN and sb % 2 == 1) else nc.vector
                    brep = bcls.tile([P, SB, TC], BF, tag="bcls")
                    crep = bcls.tile([P, SB, TC], BF, tag="bcls")
                    nc.sync.dma_start(
                        brep[:],
                        bc_scr[j, 0, ch, sb * SB * TC:(sb + 1) * SB * TC]
                        .rearrange("(s t) -> s t", s=SB)
                        .unsqueeze(0).broadcast_to((P, SB, TC)))
                    nc.sync.dma_start(
                        crep[:],
                        bc_scr[j, 1, ch, sb * SB * TC:(sb + 1) * SB * TC]
                        .rearrange("(s t) -> s t", s=SB)
                        .unsqueeze(0).broadcast_to((P, SB, TC)))
                    a_ = af32.tile([P, SB, TC + 1], FP32, tag="a")
                    for s8 in range(SB):
                        s = sb * SB + s8
                        nc.scalar.activation(a_[:, s8, 1:], dtf[db][:, cs],
                                             AF.Exp, scale=ac3[db][:, s:s + 1])
                    nc.scalar.memzero(a_[:, :, 0:1])
                    b_ = bcls.tile([P, SB, TC + 1], BF, tag="bcls")
                    nc.vector.tensor_tensor(b_[:, :, 1:], uv, brep[:],
                                            ALU.mult)
                    if ch == 0:
                        nc.vector.memset(b_[:, :, 0:1], 0.0)
                    else:
                        nc.vector.tensor_copy(b_[:, :, 0:1],
                                              hcarry[(db, sb)][:])
                    h_ = hbf.tile([P, SB, TC + 1], BF, tag="h")
                    seng.tensor_tensor_scan(
                        h_[:].rearrange("p s t -> p (s t)"),
                        a_[:].rearrange("p s t -> p (s t)"),
                        b_[:].rearrange("p s t -> p (s t)"),
                        0.0, ALU.mult, ALU.add)
                    nc.vector.tensor_copy(hcarry[(db, sb)][:],
                                          h_[:, :, TC:TC + 1])
                    hcm = bcls.tile([P, SB, TC], BF, tag="bcls")
                    nc.vector.tensor_tensor(hcm[:], h_[:, :, 1:], crep[:],
                                            ALU.mult)
                    # pairwise tree-sum over the SB states (contiguous, 2x)
                    t2 = sm.tile([P, 2, TC], BF, tag="t2")
                    nc.vector.tensor_tensor(t2[:], hcm[:, 0:2, :],
                                            hcm[:, 2:4, :], ALU.add)
                    ysb = sm.tile([P, TC], BF, tag=f"ysb{sb}")
                    nc.vector.tensor_tensor(ysb[:], t2[:, 0, :], t2[:, 1, :],
                                            ALU.add)
                    ysbs.append(ysb)
                y01 = sm.tile([P, TC], BF, tag="y01")
                nc.vector.tensor_tensor(y01[:], ysbs[0][:], ysbs[1][:], ALU.add)
                y23 = sm.tile([P, TC], BF, tag="y23")
                nc.vector.tensor_tensor(y23[:], ysbs[2][:], ysbs[3][:], ALU.add)
                y0 = sm.tile([P, TC], FP32, tag="y0")
                nc.vector.tensor_tensor(y0[:], y01[:], y23[:], ALU.add)
                # y = ys + xs*D -> bf16 -> DRAM
                nc.vector.scalar_tensor_tensor(y0[:], xs[db][:, cs],
                                               dsk3[db][:], y0[:],
                                               ALU.mult, ALU.add)
                ybf = sm.tile([P, TC], BF, tag="ybf")
                nc.scalar.copy(ybf[:], y0[:])
                nc.sync.dma_start(y_scr[j, db, :, cs], ybf[:])

    # ================= E: gate + out-proj =================
    for j in range(NB):
        for n in range(NN):
            ns = slice(n * NSZ, (n + 1) * NSZ)
            ygs = []
            for db in range(ND):
                zt = smE.tile([P, NSZ], BF, tag="ze", bufs=3)
                nc.sync.dma_start(zt[:], z_scr[j, db, :, ns])
                sgz = smE.tile([P, NSZ], BF, tag="sgz", bufs=3)
                nc.scalar.activation(sgz[:], zt[:], AF.Sigmoid)
                yt = smE.tile([P, NSZ], BF, tag="ye", bufs=3)
                nc.sync.dma_start(yt[:], y_scr[j, db, :, ns])
                nc.gpsimd.tensor_tensor(yt[:], yt[:], zt[:], ALU.mult)
                nc.gpsimd.tensor_tensor(yt[:], yt[:], sgz[:], ALU.mult)
                ygs.append(yt)
            for m in range(2):
                msz = P if m == 0 else C_ - P
                mm = slice(m * P, m * P + msz)
                pso = pp2.tile([msz, NSZ], FP32, tag="pso")
                for db in range(ND):
                    nc.tensor.matmul(pso[:], wo3[db][:, mm], ygs[db][:],
                                     start=(db == 0), stop=(db == ND - 1))
                ot = smE.tile([msz, NSZ], FP32, tag="oe")
                nc.scalar.copy(ot[:], pso[:])
                nc.sync.dma_start(outT[j, mm, ns], ot[:])


# ---------------- host side ----------------

_CACHE = {}
PROFILE = False
PROFILE_KW = {}


def _get_nc():
    if "nc" not in _CACHE:
        _CACHE["nc"] = build_nc()
    return _CACHE["nc"]


def _permute_toks(x, idx):
    """x: [C, H, W] fp32 -> 4 direction token maps, each [C, L] (c-major)."""
    c = x.shape[0]
    row = x.reshape(c, -1)
    col = x.transpose(0, 2, 1).reshape(c, -1)
    diag = row[:, idx]
    anti = x[:, :, ::-1].reshape(c, -1)[:, idx]
    return [row, col, diag, anti]


def _unpermute(outs, inv_idx, h, w):
    """outs: list of 4 [C, L] -> sum of un-permuted direction outputs."""
    c = outs[0].shape[0]
    row_f = outs[0].reshape(c, h, w)
    col_f = outs[1].reshape(c, w, h).transpose(0, 2, 1)
    diag_f = outs[2][:, inv_idx].reshape(c, h, w)
    anti_f = outs[3][:, inv_idx].reshape(c, h, w)[:, :, ::-1]
    return row_f + col_f + diag_f + anti_f


def _pack_wx(wx):
    """Pad W_x columns so dt/B/C rows land at PSUM partitions 0/32/64."""
    out = np.zeros((DI, 96), np.float32)
    out[:, 0:DTR] = wx[:, 0:DTR]
    out[:, 32:32 + DS] = wx[:, DTR:DTR + DS]
    out[:, 64:64 + DS] = wx[:, DTR + DS:]
    return out.astype(BF16)


def kernel(x, W_in, conv_w, conv_b, W_x, W_dt, b_dt, A_log, D_skip, W_out,
           idx, inv_idx):
    x = np.asarray(x, np.float32)
    idx = np.asarray(idx, np.int32)
    inv_idx = np.asarray(inv_idx, np.int32)
    A = -np.exp(np.asarray(A_log, np.float32))        # [4, DI, DS]
    conv_b = np.asarray(conv_b, np.float32)
    b_dt = np.asarray(b_dt, np.float32)
    D_skip = np.asarray(D_skip, np.float32)

    nc = _get_nc()
    in_maps = []
    for core in range(N_CORES):
        d = core // 2      # direction
        bh = core % 2      # batch half
        toks = np.empty((NB, C_, L), BF16)
        for jb in range(NB):
            b = bh * NB + jb
            toks[jb] = _permute_toks(x[b], idx)[d].astype(BF16)
        in_maps.append(dict(
            tokT=toks,
            Win=np.asarray(W_in[d], np.float32).astype(BF16),
            convw=np.ascontiguousarray(np.asarray(conv_w[d], np.float32)),
            convb=np.ascontiguousarray(conv_b[d].reshape(DI, 1)),
            Wx=_pack_wx(np.asarray(W_x[d], np.float32)),
            Wdt=np.asarray(W_dt[d], np.float32).astype(BF16),
            bdt=np.ascontiguousarray(b_dt[d].reshape(DI, 1)),
            Acoef=np.ascontiguousarray(A[d]),
            Dsk=np.ascontiguousarray(D_skip[d].reshape(DI, 1)),
            Wout=np.asarray(W_out[d], np.float32).astype(BF16),
        ))

    res = run_bass_kernel_spmd(nc, in_maps, list(range(N_CORES)),
                               trace=PROFILE, **PROFILE_KW)
    _CACHE["last_exec_ns"] = res.exec_time_ns
    outs = res.results

    # gather: per batch b, the 4 direction outputs live on cores d*2 + b//2
    acc = np.zeros((B_, C_, H_, W_), np.float32)
    for b in range(B_):
        bh, jb = b // NB, b % NB
        douts = [np.asarray(outs[d * 2 + bh]["outT"][jb], np.float32)
                 for d in range(4)]
        acc[b] = _unpermute(douts, inv_idx, H_, W_)
    gate = 1.0 / (1.0 + np.exp(-0.25 * acc))
    return x * gate


# revision 16
# speedup vs baseline: 1.1719x; 1.0604x over previous
"""AxialCrossMamba Trainium2 kernel.

Sharding: 8 cores = 4 directions x 2 batch-halves. Each core runs one
direction's Mamba block (its own weights) over two batches. Host does the
direction permutations (row/col/diag/anti, c-major [C, L] token layouts),
and the final 4-direction sigmoid gate.

Device pipeline per (job = one batch):
  in-proj matmul (bf16 PE) -> causal depthwise conv (DVE) -> silu (ACT)
  -> x-proj/dt matmuls (PE) -> softplus w/ fp32 bias (ACT)
  -> selective scan: a = exp(dt*A) per-state-column ACT activations (fp32),
     b = u*B (bf16), tensor_tensor_scan over flattened (s,t) with
     boundary-reset columns, h*C + strided reduce over s
  -> y = ys + xs*D, gate silu(z), out-proj matmul (bf16 PE).
"""

import sys

for _p in ("/opt/trn_rl_repo", "/root/.axon_site/_ro/trn_rl_repo"):
    if _p not in sys.path:
        sys.path.insert(0, _p)

from contextlib import ExitStack

import numpy as np
import ml_dtypes

import concourse.bass as bass
from concourse import bacc
import concourse.mybir as mybir
import concourse.tile as tile
from concourse.bass_utils import run_bass_kernel_spmd

BF16 = ml_dtypes.bfloat16

# Problem constants
B_, C_, H_, W_ = 4, 192, 64, 64
L = H_ * W_          # 4096 tokens
DS, DC = 16, 4       # d_state, d_conv
DI = 2 * C_          # 384 d_inner
DTR = (C_ + 15) // 16  # 12 dt_rank
NB = 2               # batches per core
ND = DI // 128       # 3 d-blocks
N_CORES = 8

GP_SCAN = False

AF = mybir.ActivationFunctionType
ALU = mybir.AluOpType
FP32 = mybir.dt.float32
BF = mybir.dt.bfloat16


def build_nc(L=L, TC=512, SB=4):
    """Build the SPMD single-core program (identical on all 8 cores)."""
    nc = bacc.Bacc("TRN2", debug=False)

    # ---- DRAM I/O ----
    tokT = nc.dram_tensor("tokT", [NB, C_, L], BF, kind="ExternalInput").ap()
    Win = nc.dram_tensor("Win", [C_, 2 * DI], BF, kind="ExternalInput").ap()
    convw = nc.dram_tensor("convw", [DI, DC], FP32, kind="ExternalInput").ap()
    convb = nc.dram_tensor("convb", [DI, 1], FP32, kind="ExternalInput").ap()
    Wx = nc.dram_tensor("Wx", [DI, 96], BF, kind="ExternalInput").ap()
    Wdt = nc.dram_tensor("Wdt", [DTR, DI], BF, kind="ExternalInput").ap()
    bdt = nc.dram_tensor("bdt", [DI, 1], FP32, kind="ExternalInput").ap()
    Acoef = nc.dram_tensor("Acoef", [DI, DS], FP32, kind="ExternalInput").ap()
    Dsk = nc.dram_tensor("Dsk", [DI, 1], FP32, kind="ExternalInput").ap()
    Wout = nc.dram_tensor("Wout", [DI, C_], BF, kind="ExternalInput").ap()
    outT = nc.dram_tensor("outT", [NB, C_, L], FP32, kind="ExternalOutput").ap()
    # scratch
    z_scr = nc.dram_tensor("z_scr", [NB, ND, 128, L], BF, kind="Internal").ap()
    y_scr = nc.dram_tensor("y_scr", [NB, ND, 128, L], BF, kind="Internal").ap()
    bc_scr = nc.dram_tensor("bc_scr", [NB, 2, L // TC, DS * TC], BF, kind="Internal").ap()

    io = dict(tokT=tokT, Win=Win, convw=convw, convb=convb, Wx=Wx, Wdt=Wdt,
              bdt=bdt, Acoef=Acoef, Dsk=Dsk, Wout=Wout, outT=outT,
              z_scr=z_scr, y_scr=y_scr, bc_scr=bc_scr)
    with tile.TileContext(nc) as tc:
        with ExitStack() as ctx:
            _emit(ctx, tc, nc, io, L=L, TC=TC, SB=SB)
    nc.compile()
    return nc


def _emit(ctx, tc, nc, io, *, L, TC, SB):
    tokT, Win, convw, convb, Wx, Wdt, bdt = (
        io["tokT"], io["Win"], io["convw"], io["convb"], io["Wx"], io["Wdt"],
        io["bdt"])
    Acoef, Dsk, Wout, outT = io["Acoef"], io["Dsk"], io["Wout"], io["outT"]
    z_scr, y_scr, bc_scr = io["z_scr"], io["y_scr"], io["bc_scr"]

    P = 128
    NCH = L // TC          # t-chunks
    NSB = DS // SB         # s-blocks
    NN = max(1, L // 512)  # matmul n-chunks
    NSZ = L // NN

    # ---- pools ----
    wp = ctx.enter_context(tc.tile_pool(name="weights", bufs=1))
    big = ctx.enter_context(tc.tile_pool(name="big", bufs=4))    # bf16 [128,L]
    fxf = ctx.enter_context(tc.tile_pool(name="fxf", bufs=1))    # fp32 conv acc
    af32 = ctx.enter_context(tc.tile_pool(name="af32", bufs=2))  # fp32 scan a
    hbf = ctx.enter_context(tc.tile_pool(name="hbf", bufs=2))    # bf16 scan h
    bcls = ctx.enter_context(tc.tile_pool(name="bcls", bufs=6))  # bf16 scan misc
    dtp = ctx.enter_context(tc.tile_pool(name="dtp", bufs=1))    # dt bf16 resident
    xsp = ctx.enter_context(tc.tile_pool(name="xsp", bufs=1))    # xs bf16 resident
    sm = ctx.enter_context(tc.tile_pool(name="small", bufs=2))
    smE = ctx.enter_context(tc.tile_pool(name="smallE", bufs=2))
    pp = ctx.enter_context(tc.tile_pool(name="psum", bufs=4, space="PSUM"))
    pp2 = ctx.enter_context(tc.tile_pool(name="psum2", bufs=2, space="PSUM"))

    # ---- load weights ----
    win0 = wp.tile([P, 2 * DI], BF, tag="win0")
    win1 = wp.tile([C_ - P, 2 * DI], BF, tag="win1")
    nc.sync.dma_start(win0[:], Win[0:P, :])
    nc.sync.dma_start(win1[:], Win[P:C_, :])
    wdt_full = wp.tile([DTR, DI], BF, tag="wdt")
    nc.sync.dma_start(wdt_full[:], Wdt[:])
    wxs, cw3, cb3, bdt3, ac3, dsk3, wo3 = [], [], [], [], [], [], []
    for db in range(ND):
        r = slice(db * P, (db + 1) * P)
        w1 = wp.tile([P, 96], BF, tag=f"wx{db}")
        nc.sync.dma_start(w1[:], Wx[r, :]); wxs.append(w1)
        w2 = wp.tile([P, DC], FP32, tag=f"cw{db}")
        nc.sync.dma_start(w2[:], convw[r, :]); cw3.append(w2)
        w3 = wp.tile([P, 1], FP32, tag=f"cb{db}")
        nc.sync.dma_start(w3[:], convb[r, :]); cb3.append(w3)
        w4 = wp.tile([P, 1], FP32, tag=f"bdt{db}")
        nc.sync.dma_start(w4[:], bdt[r, :]); bdt3.append(w4)
        w5 = wp.tile([P, DS], FP32, tag=f"ac{db}")
        nc.sync.dma_start(w5[:], Acoef[r, :]); ac3.append(w5)
        w6 = wp.tile([P, 1], FP32, tag=f"dsk{db}")
        nc.sync.dma_start(w6[:], Dsk[r, :]); dsk3.append(w6)
        w7 = wp.tile([P, C_], BF, tag=f"wo{db}")
        nc.sync.dma_start(w7[:], Wout[r, :]); wo3.append(w7)

    for j in range(NB):
        # ================= A: in-proj (+ conv interleaved) =================
        tok0 = big.tile([P, L], BF, tag="big")
        tok1 = big.tile([C_ - P, L], BF, tag="big")
        nc.sync.dma_start(tok0[:], tokT[j, 0:P, :])
        nc.sync.dma_start(tok1[:], tokT[j, P:C_, :])

        xs = []
        for m in range(2 * DI // P):   # M-blocks of xz^T; 0..2 -> xi, 3..5 -> z
            if m < ND:
                xi = big.tile([P, L + DC], BF, tag="big")
                nc.scalar.memzero(xi[:, 0:DC])
            mm = slice(m * P, (m + 1) * P)
            for n in range(NN):
                ns = slice(n * NSZ, (n + 1) * NSZ)
                ps = pp.tile([P, NSZ], FP32, tag="ps")
                nc.tensor.matmul(ps[:], win0[:, mm], tok0[:, ns],
                                 start=True, stop=False)
                nc.tensor.matmul(ps[:], win1[:, mm], tok1[:, ns],
                                 start=False, stop=True)
                if m < ND:
                    nc.scalar.copy(xi[:, DC + n * NSZ: DC + (n + 1) * NSZ],
                                   ps[:])
                else:
                    zt = smE.tile([P, NSZ], BF, tag="ztmp")
                    nc.vector.tensor_copy(zt[:], ps[:])
                    nc.sync.dma_start(z_scr[j, m - ND, :, ns], zt[:])
            if m < ND:
                # conv + silu for this d-block; frees xi slot for the next one
                db = m
                x_ = xsp.tile([P, L], BF, tag=f"xs{db}")
                xf = fxf.tile([P, L], FP32, tag="xf")
                # xc[t] = sum_k w[k]*tok[t-3+k]; token t sits at xi col t+DC
                nc.vector.tensor_scalar_mul(xf[:], xi[:, 1:1 + L],
                                            cw3[db][:, 0:1])
                for k in range(1, DC):
                    nc.vector.scalar_tensor_tensor(
                        xf[:], xi[:, 1 + k:1 + k + L], cw3[db][:, k:k + 1],
                        xf[:], ALU.mult, ALU.add)
                sgt = big.tile([P, L], BF, tag="big")
                nc.scalar.activation(sgt[:], xf[:], AF.Sigmoid, bias=cb3[db])
                nc.vector.scalar_tensor_tensor(x_[:], xf[:], cb3[db][:],
                                               sgt[:], ALU.add, ALU.mult)
                xs.append(x_)

        # ================= C: dbc, dt =================
        dtl = bcls.tile([DTR, L], BF, tag="dtl", bufs=1)
        for n in range(NN):
            ns = slice(n * NSZ, (n + 1) * NSZ)
            psd = pp2.tile([96, NSZ], FP32, tag="psd")
            for db in range(ND):
                nc.tensor.matmul(psd[:], wxs[db][:], xs[db][:, ns],
                                 start=(db == 0), stop=(db == ND - 1))
            nc.scalar.copy(dtl[:, ns], psd[0:DTR, :])
            bt = smE.tile([DS, NSZ], BF, tag="bct")
            ct = smE.tile([DS, NSZ], BF, tag="bct")
            nc.vector.tensor_copy(bt[:], psd[32:32 + DS, :])
            nc.vector.tensor_copy(ct[:], psd[64:64 + DS, :])
            for r in range(max(1, NSZ // TC)):
                rs = slice(r * TC, (r + 1) * TC)
                nc.sync.dma_start(
                    bc_scr[j, 0, n * (NSZ // TC) + r]
                    .rearrange("(s t) -> s t", s=DS), bt[:, rs])
                nc.sync.dma_start(
                    bc_scr[j, 1, n * (NSZ // TC) + r]
                    .rearrange("(s t) -> s t", s=DS), ct[:, rs])
        dtf = []
        for db in range(ND):
            d_ = dtp.tile([P, L], BF, tag=f"dt{db}")
            esp = fxf.tile([P, L], FP32, tag="xf")
            for n in range(NN):
                ns = slice(n * NSZ, (n + 1) * NSZ)
                psm = pp.tile([P, NSZ], FP32, tag="ps")
                nc.tensor.matmul(psm[:], wdt_full[:, db * P:(db + 1) * P],
                                 dtl[:, ns], start=True, stop=True)
                nc.scalar.activation(esp[:, ns], psm[:], AF.Exp, bias=bdt3[db])
            nc.scalar.activation(d_[:], esp[:], AF.Ln, bias=1.0)
            dtf.append(d_)

        # ================= D: selective scan =================
        hcarry = {}
        for db in range(ND):
            for sb in range(NSB):
                hcarry[(db, sb)] = sm.tile([P, SB, 1], FP32, name=f"carry{db}{sb}",
                                           tag=f"carry{db}_{sb}", bufs=1)
        for ch in range(NCH):
            cs = slice(ch * TC, (ch + 1) * TC)
            for db in range(ND):
                uch = sm.tile([P, TC], BF, tag="uch")
                nc.vector.tensor_tensor(uch[:], dtf[db][:, cs], xs[db][:, cs],
                                        ALU.mult)
                uv = uch[:].unsqueeze(1).broadcast_to((P, SB, TC))
                ysbs = []
                for sb in range(NSB):
                    seng = nc.gpsimd if (GP_SCAN and sb % 2 == 1) else nc.vector
                    brep = bcls.tile([P, SB, TC], BF, tag="bcls")
                    crep = bcls.tile([P, SB, TC], BF, tag="bcls")
                    nc.sync.dma_start(
                        brep[:],
                        bc_scr[j, 0, ch, sb * SB * TC:(sb + 1) * SB * TC]
                        .rearrange("(s t) -> s t", s=SB)
                        .unsqueeze(0).broadcast_to((P, SB, TC)))
                    nc.sync.dma_start(
                        crep[:],
                        bc_scr[j, 1, ch, sb * SB * TC:(sb + 1) * SB * TC]
                        .rearrange("(s t) -> s t", s=SB)
                        .unsqueeze(0).broadcast_to((P, SB, TC)))
                    a_ = af32.tile([P, SB, TC + 1], FP32, tag="a")
                    for s8 in range(SB):
                        s = sb * SB + s8
                        nc.scalar.activation(a_[:, s8, 1:], dtf[db][:, cs],
                                             AF.Exp, scale=ac3[db][:, s:s + 1])
                    nc.scalar.memzero(a_[:, :, 0:1])
                    b_ = bcls.tile([P, SB, TC + 1], BF, tag="bcls")
                    nc.vector.tensor_tensor(b_[:, :, 1:], uv, brep[:],
                                            ALU.mult)
                    if ch == 0:
                        nc.vector.memset(b_[:, :, 0:1], 0.0)
                    else:
                        nc.vector.tensor_copy(b_[:, :, 0:1],
                                              hcarry[(db, sb)][:])
                    h_ = hbf.tile([P, SB, TC + 1], BF, tag="h")
                    seng.tensor_tensor_scan(
                        h_[:].rearrange("p s t -> p (s t)"),
                        a_[:].rearrange("p s t -> p (s t)"),
                        b_[:].rearrange("p s t -> p (s t)"),
                        0.0, ALU.mult, ALU.add)
                    nc.vector.tensor_copy(hcarry[(db, sb)][:],
                                          h_[:, :, TC:TC + 1])
                    hcm = bcls.tile([P, SB, TC], BF, tag="bcls")
                    nc.vector.tensor_tensor(hcm[:], h_[:, :, 1:], crep[:],
                                            ALU.mult)
                    # pairwise tree-sum over the SB states (contiguous, 2x)
                    t2 = sm.tile([P, 2, TC], BF, tag="t2")
                    nc.vector.tensor_tensor(t2[:], hcm[:, 0:2, :],
                                            hcm[:, 2:4, :], ALU.add)
                    ysb = sm.tile([P, TC], BF, tag=f"ysb{sb}")
                    nc.vector.tensor_tensor(ysb[:], t2[:, 0, :], t2[:, 1, :],
                                            ALU.add)
                    ysbs.append(ysb)
                y01 = sm.tile([P, TC], BF, tag="y01")
                nc.vector.tensor_tensor(y01[:], ysbs[0][:], ysbs[1][:], ALU.add)
                y23 = sm.tile([P, TC], BF, tag="y23")
                nc.vector.tensor_tensor(y23[:], ysbs[2][:], ysbs[3][:], ALU.add)
                y0 = sm.tile([P, TC], FP32, tag="y0")
                nc.vector.tensor_tensor(y0[:], y01[:], y23[:], ALU.add)
                # y = ys + xs*D -> bf16 -> DRAM
                nc.vector.scalar_tensor_tensor(y0[:], xs[db][:, cs],
                                               dsk3[db][:], y0[:],
                                               ALU.mult, ALU.add)
                ybf = sm.tile([P, TC], BF, tag="ybf")
                nc.scalar.copy(ybf[:], y0[:])
                nc.sync.dma_start(y_scr[j, db, :, cs], ybf[:])

    # ================= E: gate + out-proj =================
    for j in range(NB):
        for n in range(NN):
            ns = slice(n * NSZ, (n + 1) * NSZ)
            ygs = []
            for db in range(ND):
                zt = smE.tile([P, NSZ], BF, tag="ze", bufs=3)
                nc.sync.dma_start(zt[:], z_scr[j, db, :, ns])
                sgz = smE.tile([P, NSZ], BF, tag="sgz", bufs=3)
                nc.scalar.activation(sgz[:], zt[:], AF.Sigmoid)
                yt = smE.tile([P, NSZ], BF, tag="ye", bufs=3)
                nc.sync.dma_start(yt[:], y_scr[j, db, :, ns])
                nc.vector.tensor_tensor(yt[:], yt[:], zt[:], ALU.mult)
                nc.vector.tensor_tensor(yt[:], yt[:], sgz[:], ALU.mult)
                ygs.append(yt)
            for m in range(2):
                msz = P if m == 0 else C_ - P
                mm = slice(m * P, m * P + msz)
                pso = pp2.tile([msz, NSZ], FP32, tag="pso")
                for db in range(ND):
                    nc.tensor.matmul(pso[:], wo3[db][:, mm], ygs[db][:],
                                     start=(db == 0), stop=(db == ND - 1))
                ot = smE.tile([msz, NSZ], FP32, tag="oe")
                nc.scalar.copy(ot[:], pso[:])
                nc.sync.dma_start(outT[j, mm, ns], ot[:])


# ---------------- host side ----------------

_CACHE = {}
PROFILE = False
PROFILE_KW = {}


def _get_nc():
    if "nc" not in _CACHE:
        _CACHE["nc"] = build_nc()
    return _CACHE["nc"]


def _permute_toks(x, idx):
    """x: [C, H, W] fp32 -> 4 direction token maps, each [C, L] (c-major)."""
    c = x.shape[0]
    row = x.reshape(c, -1)
    col = x.transpose(0, 2, 1).reshape(c, -1)
    diag = row[:, idx]
    anti = x[:, :, ::-1].reshape(c, -1)[:, idx]
    return [row, col, diag, anti]


def _unpermute(outs, inv_idx, h, w):
    """outs: list of 4 [C, L] -> sum of un-permuted direction outputs."""
    c = outs[0].shape[0]
    row_f = outs[0].reshape(c, h, w)
    col_f = outs[1].reshape(c, w, h).transpose(0, 2, 1)
    diag_f = outs[2][:, inv_idx].reshape(c, h, w)
    anti_f = outs[3][:, inv_idx].reshape(c, h, w)[:, :, ::-1]
    return row_f + col_f + diag_f + anti_f


def _pack_wx(wx):
    """Pad W_x columns so dt/B/C rows land at PSUM partitions 0/32/64."""
    out = np.zeros((DI, 96), np.float32)
    out[:, 0:DTR] = wx[:, 0:DTR]
    out[:, 32:32 + DS] = wx[:, DTR:DTR + DS]
    out[:, 64:64 + DS] = wx[:, DTR + DS:]
    return out.astype(BF16)


def kernel(x, W_in, conv_w, conv_b, W_x, W_dt, b_dt, A_log, D_skip, W_out,
           idx, inv_idx):
    x = np.asarray(x, np.float32)
    idx = np.asarray(idx, np.int32)
    inv_idx = np.asarray(inv_idx, np.int32)
    A = -np.exp(np.asarray(A_log, np.float32))        # [4, DI, DS]
    conv_b = np.asarray(conv_b, np.float32)
    b_dt = np.asarray(b_dt, np.float32)
    D_skip = np.asarray(D_skip, np.float32)

    nc = _get_nc()
    in_maps = []
    for core in range(N_CORES):
        d = core // 2      # direction
        bh = core % 2      # batch half
        toks = np.empty((NB, C_, L), BF16)
        for jb in range(NB):
            b = bh * NB + jb
            toks[jb] = _permute_toks(x[b], idx)[d].astype(BF16)
        in_maps.append(dict(
            tokT=toks,
            Win=np.asarray(W_in[d], np.float32).astype(BF16),
            convw=np.ascontiguousarray(np.asarray(conv_w[d], np.float32)),
            convb=np.ascontiguousarray(conv_b[d].reshape(DI, 1)),
            Wx=_pack_wx(np.asarray(W_x[d], np.float32)),
            Wdt=np.asarray(W_dt[d], np.float32).astype(BF16),
            bdt=np.ascontiguousarray(b_dt[d].reshape(DI, 1)),
            Acoef=np.ascontiguousarray(A[d]),
            Dsk=np.ascontiguousarray(D_skip[d].reshape(DI, 1)),
            Wout=np.asarray(W_out[d], np.float32).astype(BF16),
        ))

    res = run_bass_kernel_spmd(nc, in_maps, list(range(N_CORES)),
                               trace=PROFILE, **PROFILE_KW)
    _CACHE["last_exec_ns"] = res.exec_time_ns
    outs = res.results

    # gather: per batch b, the 4 direction outputs live on cores d*2 + b//2
    acc = np.zeros((B_, C_, H_, W_), np.float32)
    for b in range(B_):
        bh, jb = b // NB, b % NB
        douts = [np.asarray(outs[d * 2 + bh]["outT"][jb], np.float32)
                 for d in range(4)]
        acc[b] = _unpermute(douts, inv_idx, H_, W_)
    gate = 1.0 / (1.0 + np.exp(-0.25 * acc))
    return x * gate
